# revision 6
# baseline (speedup 1.0000x reference)
"""Trainium2 Bass kernel for nn_CriticNetwork (LSTM T=3, D=18, H=64 + MLP 64->32->1).

v2: pure data parallel over 8 cores (65536 batch each), 64 j-iterations of
1024 elements (A-half 512 on partitions 0:64, B-half on 64:128).

TensorE: x-projections run as 8 concurrent subarray tiles (K=20, M=64,
N=512) at tile positions (32r, 64c) -- r selects the gate (i,o,g,f), c the
batch half -- so each step's entire x-projection costs ~1 matmul slot. The
x data is replicated on 4 partition groups for this. h-projections are
K=128 blockdiag matmuls accumulating into the same PSUM slots (these also
keep the PE's HAM activity monitor warm -> 2.4 GHz streaming).

PSUM tags: Pio [128,1024] (i,o slots, bufs=2), Pg [128,512], Pf [128,512].
ScalarE: one sigmoid act per step over [i,o]; MLP relu; value copy-out.
VectorE (custom ops): ig = tanh5(g_pre)*sigma_i straight from PSUM
(deg-5 odd tanh fit on [-2.2,2.2]); fc = 2*sigma(f_pre)*c_prev from PSUM
(sigscale); h = sigma_o*tanh5(c) as pair-ops over two j's. The running
c-scale ladder k=1/4 -> 1/2 -> 1 makes sigscale's factor-2 exact.
GpSimd: the c = ig + fc adds.

MLP for block b runs during block b+1's phase 1 (Pf tag is free there);
b2 is added on the host after gathering.
"""
import os
import numpy as np
import ml_dtypes

import concourse.bacc as bacc
import concourse.bass as bass
import concourse.mybir as mybir
import concourse.tile as tile
from concourse import bass_utils

F32 = mybir.dt.float32
BF16 = mybir.dt.bfloat16
AF = mybir.ActivationFunctionType

NCORES = 8
W = 512
ITERS = int(os.environ.get("K_ITERS", "64"))
BCORE = ITERS * 2 * W          # 65536 at full size
BATCH = BCORE * NCORES
BLK = int(os.environ.get("K_BLK", "8"))
NBLK = ITERS // BLK
BW = BLK * W                   # 4096: A(or B)-half width per block
XW2 = 2 * ITERS * W            # 65536: x columns per core

STATE_DIM, SEQ_LEN, HIDDEN, MLP_HIDDEN = 18, 3, 64, 32
KX = STATE_DIM + 2             # x rows + 2 ones rows (bias hi/lo)

# tanh deg-5 odd minimax on [-1.35,1.35] (for tanh(c)), maxerr 1.9e-3
T5H = (0.9892881449596069, -0.2766816650158005, 0.049318986652270407)
# tanh(x/2) deg-5 odd minimax on [-3.2,3.2] (for 2*sigmoid(f)), maxerr 4.2e-3
TF5 = (0.4896413693764065, -0.031307111156723334, 0.0011471116956348172)
# tanh deg-5 odd minimax on [-2.2,2.2] (for tanh(g_pre)), maxerr 1.5e-2
T25 = (0.9414658776605921, -0.1885272303414886, 0.017991445957406323)

KLAD = (0.25, 0.5, 1.0)        # running c-scale per step


def _hk(k):
    """tanh(c) evaluated on chat = k*c: chat*(c0/k + c1*chat^2/k^3 + ...)."""
    return (T5H[0] / k, T5H[1] / k ** 3, T5H[2] / k ** 5)


def _tk(k):
    """k * tanh5(g_pre) coefficients."""
    return (T25[0] * k, T25[1] * k, T25[2] * k)


_ops = None


def get_ops():
    """Register the two custom DVE ops (idempotent)."""
    global _ops
    if _ops is not None:
        return _ops
    import concourse.dve_ops as dve_ops
    from concourse.dve_spec import Spec, Src0, Src1, C0, C1, C2, One, sq, lower
    from concourse.dve_uop import DveOpSpec

    def make(name, body, ref):
        for op in dve_ops.OPS:
            if op.name == name:
                return op
        spec = Spec(body=body, reference=ref)
        if name not in dve_ops._SUB_OPCODE_FOR_NAME:
            dve_ops._SUB_OPCODE_FOR_NAME[name] = (
                max(dve_ops._SUB_OPCODE_FOR_NAME.values()) + 1)
        shas = {}
        for ver in ("v3", "v4"):
            try:
                s = DveOpSpec(name=name, opcode=dve_ops.get_dve_sub_opcode(name),
                              uops=lower(spec, ver=ver), rd1_en=True)
                shas[ver] = s.sha(ver)
            except Exception:
                pass
        op = dve_ops.DveOp(name, spec, subdim=False, uops_sha=shas)
        dve_ops.OPS.append(op)
        return op

    u = sq(Src0)
    poly = ((C2 * u + C1) * u + C0) * Src0

    def _ref_tm(in0, in1, s0, s1, imm2):
        x = in0.astype(np.float32)
        uu = x * x
        return ((imm2 * uu + s1) * uu + s0) * x * in1.astype(np.float32)

    def _ref_fs(in0, in1, s0, s1, imm2):
        x = in0.astype(np.float32)
        uu = x * x
        return (((imm2 * uu + s1) * uu + s0) * x + 1.0) * in1.astype(np.float32)

    tanhmul = make("TANHMUL5_ANT", poly * Src1, _ref_tm)
    sigscale = make("SIGSCALE5_ANT", (poly + One) * Src1, _ref_fs)
    _ops = (tanhmul, sigscale)
    return _ops


def build_bass():
    tanhmul, sigscale = get_ops()
    nc = bacc.Bacc("TRN2", target_bir_lowering=False, debug=False)

    W2_ = 2 * W

    xs_d = [nc.dram_tensor(f"xs{t}", [KX, XW2], BF16, kind="ExternalInput").ap()
            for t in range(SEQ_LEN)]
    wxq_d = nc.dram_tensor("wxq", [128, 64], BF16, kind="ExternalInput").ap()
    wh_d = {g: nc.dram_tensor(f"wh{g}", [128, 128], BF16,
                              kind="ExternalInput").ap() for g in "iogf"}
    w1_d = nc.dram_tensor("w1e", [128, 64], BF16, kind="ExternalInput").ap()
    w2_d = nc.dram_tensor("w2e", [64, 2], BF16, kind="ExternalInput").ap()
    b1_d = nc.dram_tensor("b1e", [64, 1], F32, kind="ExternalInput").ap()
    out_d = nc.dram_tensor("out", [2 * NBLK, BW], F32, kind="ExternalOutput").ap()

    # gate -> row-group of wxq / x quad-replication
    GR = {"i": 0, "o": 1, "g": 2, "f": 3}

    with tile.TileContext(nc) as tc:
        with tc.tile_pool(name="const", bufs=1) as cst, \
             tc.tile_pool(name="xt", bufs=2) as xtp, \
             tc.tile_pool(name="sg", bufs=4) as sgp, \
             tc.tile_pool(name="cw", bufs=6) as cwp, \
             tc.tile_pool(name="c3w", bufs=3) as c3p, \
             tc.tile_pool(name="hw", bufs=6) as hwp, \
             tc.tile_pool(name="h3", bufs=8) as h3p, \
             tc.tile_pool(name="igfc", bufs=3) as igp, \
             tc.tile_pool(name="zr", bufs=3) as zrp, \
             tc.tile_pool(name="vo", bufs=3) as vop, \
             tc.tile_pool(name="P4", bufs=2, space="PSUM") as p4p:

            wxq = cst.tile([128, 64], BF16)
            nc.sync.dma_start(wxq[:], wxq_d[:])
            wh = {}
            for g in "iogf":
                wh[g] = cst.tile([128, 128], BF16, name=f"wh{g}")
                nc.sync.dma_start(wh[g][:], wh_d[g][:])
            w1e = cst.tile([128, 64], BF16)
            nc.sync.dma_start(w1e[:], w1_d[:])
            w2e = cst.tile([64, 2], BF16)
            nc.sync.dma_start(w2e[:], w2_d[:])
            b1e = cst.tile([64, 1], F32)
            nc.sync.dma_start(b1e[:], b1_d[:])

            def x_mms(xt, j, gates, P4, stop):
                """x-projection for one j: concurrent subarray tiles."""
                cx = j * W
                dst = {"i": P4[:, 0:W], "o": P4[:, W:W2_],
                       "g": P4[:, W2_:3 * W], "f": P4[:, 3 * W:4 * W]}
                for gate in gates:
                    r = GR[gate]
                    for c in (0, 1):
                        nc.tensor.matmul(
                            dst[gate][64 * c:64 * c + 64, :],
                            wxq[32 * r:32 * r + KX, 0:64],
                            xt[32 * r:32 * r + KX, c * BW + cx:c * BW + cx + W],
                            start=True, stop=stop,
                            tile_position=(32 * r, 64 * c),
                            skip_group_check=True)

            def h_mms(hpair, j, P4):
                """h-projection accumulate for one j (K=128 blockdiag)."""
                rhs = hpair[:, (j % 2) * W:(j % 2) * W + W]
                for gate, dst in (("i", P4[:, 0:W]), ("o", P4[:, W:W2_]),
                                  ("g", P4[:, W2_:3 * W]),
                                  ("f", P4[:, 3 * W:4 * W])):
                    nc.tensor.matmul(dst, wh[gate][:, :], rhs,
                                     start=False, stop=True,
                                     skip_group_check=True)

            def sig_act(Pio, sgt, j):
                """sigma over [i,o] -> sg pair tile strided halves."""
                p = j % 2
                src = Pio[:, 0:W2_].rearrange("p (g w) -> p g w", g=2, w=W)
                dst = sgt.rearrange("p (g j w) -> p g j w", g=2, j=2, w=W)[:, :, p, :]
                nc.scalar.activation(dst, src, AF.Sigmoid)

            def mlp(h3pair, blk, j, P4):
                """MLP for (prev) block blk, iteration j, in P4's free f-slot."""
                Pz = P4[:, 3 * W:4 * W]
                nc.tensor.matmul(Pz[0:64, :], w1e[:, :],
                                 h3pair[:, (j % 2) * W:(j % 2) * W + W],
                                 start=True, stop=True, skip_group_check=True)
                zrt = zrp.tile([64, W], BF16, tag="zr")
                nc.scalar.activation(zrt[:], Pz[0:64, :], AF.Relu, bias=b1e[:])
                nc.tensor.matmul(Pz[96:98, :], w2e[:, :], zrt[:],
                                 start=True, stop=True, tile_position=(0, 96),
                                 skip_group_check=True)
                vot = vop.tile([2, W], F32, tag="vo")
                nc.scalar.copy(vot[:], Pz[96:98, :])
                nc.sync.dma_start(
                    out_d[2 * blk:2 * blk + 2, j * W:(j + 1) * W], vot[:])

            h3_prev = None
            for blk in range(NBLK):
                xt = []
                for t in range(SEQ_LEN):
                    xb = xtp.tile([128, 2 * BW], BF16, tag=f"x{t}")
                    for r in range(4):
                        nc.sync.dma_start(
                            xb[32 * r:32 * r + KX, :],
                            xs_d[t][:, blk * 2 * BW:(blk + 1) * 2 * BW])
                    xt.append(xb)

                # ---------- phase 1: step 1 (h0 = c0 = 0) + prev-block MLP ----
                c1s, h1s = [], []
                for j in range(BLK):
                    Pio = p4p.tile([128, 4 * W], F32, tag="P4", name="Pio1")
                    x_mms(xt[0], j, "iog", Pio, stop=True)
                    if h3_prev is not None:
                        mlp(h3_prev[j // 2], blk - 1, j, Pio)
                    if j % 2 == 0:
                        sgt = sgp.tile([128, 4 * W], BF16, tag="sg", name="sg1")
                        c1t = cwp.tile([128, W2_], BF16, tag="c1", name="c1t")
                    sig_act(Pio, sgt, j)
                    p = j % 2
                    nc.vector._custom_dve(
                        tanhmul, out=c1t[:, p * W:p * W + W],
                        in0=Pio[:, W2_:3 * W],
                        in1=sgt[:, p * W:p * W + W],
                        s0=_tk(KLAD[0])[0], s1=_tk(KLAD[0])[1],
                        imm2=_tk(KLAD[0])[2])
                    if j % 2 == 1:
                        h1t = hwp.tile([128, W2_], BF16, tag="h1", name="h1t")
                        hco = _hk(KLAD[0])
                        nc.vector._custom_dve(
                            tanhmul, out=h1t[:], in0=c1t[:],
                            in1=sgt[:, W2_:4 * W], s0=hco[0], s1=hco[1],
                            imm2=hco[2])
                        c1s.append(c1t)
                        h1s.append(h1t)

                # ---------- phase 2: step 2 ----------
                c2s, h2s = [], []
                for j in range(BLK):
                    Pio = p4p.tile([128, 4 * W], F32, tag="P4", name="Pio2")
                    x_mms(xt[1], j, "iogf", Pio, stop=False)
                    h_mms(h1s[j // 2], j, Pio)
                    if j % 2 == 0:
                        sgt = sgp.tile([128, 4 * W], BF16, tag="sg", name="sg2")
                        c2t = cwp.tile([128, W2_], BF16, tag="c2", name="c2t")
                    p = j % 2
                    fct = igp.tile([128, W], BF16, tag="fc")
                    nc.vector._custom_dve(
                        sigscale, out=fct[:], in0=Pio[:, 3 * W:4 * W],
                        in1=c1s[j // 2][:, p * W:p * W + W],
                        s0=TF5[0], s1=TF5[1], imm2=TF5[2])
                    sig_act(Pio, sgt, j)
                    igt = igp.tile([128, W], BF16, tag="ig")
                    nc.vector._custom_dve(
                        tanhmul, out=igt[:], in0=Pio[:, W2_:3 * W],
                        in1=sgt[:, p * W:p * W + W],
                        s0=_tk(KLAD[1])[0], s1=_tk(KLAD[1])[1],
                        imm2=_tk(KLAD[1])[2])
                    nc.gpsimd.tensor_add(c2t[:, p * W:p * W + W], igt[:], fct[:])
                    if j % 2 == 1:
                        h2t = hwp.tile([128, W2_], BF16, tag="h2", name="h2t")
                        hco = _hk(KLAD[1])
                        nc.vector._custom_dve(
                            tanhmul, out=h2t[:], in0=c2t[:],
                            in1=sgt[:, W2_:4 * W], s0=hco[0], s1=hco[1],
                            imm2=hco[2])
                        c2s.append(c2t)
                        h2s.append(h2t)

                # ---------- phase 3: step 3 ----------
                h3_cur = []
                for j in range(BLK):
                    Pio = p4p.tile([128, 4 * W], F32, tag="P4", name="Pio3")
                    x_mms(xt[2], j, "iogf", Pio, stop=False)
                    h_mms(h2s[j // 2], j, Pio)
                    if j % 2 == 0:
                        sgt = sgp.tile([128, 4 * W], BF16, tag="sg", name="sg3")
                        c3t = c3p.tile([128, W2_], BF16, tag="c3", name="c3t")
                    p = j % 2
                    fct = igp.tile([128, W], BF16, tag="fc")
                    nc.vector._custom_dve(
                        sigscale, out=fct[:], in0=Pio[:, 3 * W:4 * W],
                        in1=c2s[j // 2][:, p * W:p * W + W],
                        s0=TF5[0], s1=TF5[1], imm2=TF5[2])
                    sig_act(Pio, sgt, j)
                    igt = igp.tile([128, W], BF16, tag="ig")
                    nc.vector._custom_dve(
                        tanhmul, out=igt[:], in0=Pio[:, W2_:3 * W],
                        in1=sgt[:, p * W:p * W + W],
                        s0=_tk(KLAD[2])[0], s1=_tk(KLAD[2])[1],
                        imm2=_tk(KLAD[2])[2])
                    nc.gpsimd.tensor_add(c3t[:, p * W:p * W + W], igt[:], fct[:])
                    if j % 2 == 1:
                        h3t = h3p.tile([128, W2_], BF16, tag="h3", name="h3t")
                        hco = _hk(KLAD[2])
                        nc.vector._custom_dve(
                            tanhmul, out=h3t[:], in0=c3t[:],
                            in1=sgt[:, W2_:4 * W], s0=hco[0], s1=hco[1],
                            imm2=hco[2])
                        h3_cur.append(h3t)

                h3_prev = h3_cur

            # MLP for the last block
            for j in range(BLK):
                Pm = p4p.tile([128, 4 * W], F32, tag="P4", name="Pm")
                mlp(h3_prev[j // 2], NBLK - 1, j, Pm)

    nc.compile()
    return nc


def _host_prep(state_seq, W_ih, W_hh, b_ih, b_hh, W1, b1, W2, b2):
    """Build per-core input maps (host-side layout prep only)."""
    bf = ml_dtypes.bfloat16
    B = state_seq.shape[0]
    b = b_ih.astype(np.float64) + b_hh.astype(np.float64)  # [256]
    GROW = {"i": 0, "f": 64, "g": 128, "o": 192}           # torch gate order
    GR = {"i": 0, "o": 1, "g": 2, "f": 3}                  # row-group order

    wxq = np.zeros((128, 64), np.float64)
    for gate, r in GR.items():
        r0 = GROW[gate]
        Wx = W_ih[r0:r0 + 64, :].astype(np.float64)        # [64, 18]
        bg = b[r0:r0 + 64]
        b_hi = bg.astype(bf).astype(np.float64)
        b_lo = (bg - b_hi).astype(bf).astype(np.float64)
        wxq[32 * r:32 * r + STATE_DIM, :] = Wx.T
        wxq[32 * r + STATE_DIM, :] = b_hi
        wxq[32 * r + STATE_DIM + 1, :] = b_lo

    shared = {"wxq": wxq.astype(bf)}
    for gate in "iogf":
        r0 = GROW[gate]
        Wh = W_hh[r0:r0 + 64, :].astype(np.float64)        # [64, 64]
        whe = np.zeros((128, 128), np.float64)
        for half in range(2):
            o = half * 64
            whe[o:o + 64, o:o + 64] = Wh.T
        shared[f"wh{gate}"] = whe.astype(bf)

    w1e = np.zeros((128, 64), np.float64)
    w2e = np.zeros((64, 2), np.float64)
    b1e = np.zeros((64, 1), np.float32)
    for half in range(2):
        ro, co = half * 64, half * 32
        w1e[ro:ro + 64, co:co + 32] = W1.astype(np.float64).T
        w2e[co:co + 32, half] = W2[0].astype(np.float64)
        b1e[co:co + 32, 0] = b1
    shared["w1e"] = w1e.astype(bf)
    shared["w2e"] = w2e.astype(bf)
    shared["b1e"] = b1e

    # x: [KX, B_core] per step; ones rows 18,19; col = flat element index
    in_maps = []
    for cc in range(NCORES):
        lo = cc * BCORE
        m = dict(shared)
        for t in range(SEQ_LEN):
            a = np.ones((KX, BCORE), np.float32)
            a[0:STATE_DIM] = state_seq[lo:lo + BCORE, t, :].T
            m[f"xs{t}"] = a.astype(bf)
        in_maps.append(m)
    return in_maps


_cached = {}


def kernel(**inputs) -> np.ndarray:
    if "nc" not in _cached:
        _cached["nc"] = build_bass()
    nc = _cached["nc"]
    in_maps = _host_prep(**inputs)
    trace = bool(int(os.environ.get("K_TRACE", "0")))
    res = bass_utils.run_bass_kernel_spmd(nc, in_maps, core_ids=list(range(NCORES)),
                                          trace=trace)
    outs = [r["out"].reshape(-1) for r in res.results]
    _cached["last_results"] = res
    out = np.concatenate(outs).astype(np.float32)
    return out + np.float32(inputs["b2"][0])


# revision 7
# speedup vs baseline: 1.0611x; 1.0611x over previous
"""Trainium2 Bass kernel for nn_CriticNetwork (LSTM T=3, D=18, H=64 + MLP 64->32->1).

v2: pure data parallel over 8 cores (65536 batch each), 64 j-iterations of
1024 elements (A-half 512 on partitions 0:64, B-half on 64:128).

TensorE: x-projections run as 8 concurrent subarray tiles (K=20, M=64,
N=512) at tile positions (32r, 64c) -- r selects the gate (i,o,g,f), c the
batch half -- so each step's entire x-projection costs ~1 matmul slot. The
x data is replicated on 4 partition groups for this. h-projections are
K=128 blockdiag matmuls accumulating into the same PSUM slots (these also
keep the PE's HAM activity monitor warm -> 2.4 GHz streaming).

PSUM tags: Pio [128,1024] (i,o slots, bufs=2), Pg [128,512], Pf [128,512].
ScalarE: one sigmoid act per step over [i,o]; MLP relu; value copy-out.
VectorE (custom ops): ig = tanh5(g_pre)*sigma_i straight from PSUM
(deg-5 odd tanh fit on [-2.2,2.2]); fc = 2*sigma(f_pre)*c_prev from PSUM
(sigscale); h = sigma_o*tanh5(c) as pair-ops over two j's. The running
c-scale ladder k=1/4 -> 1/2 -> 1 makes sigscale's factor-2 exact.
GpSimd: the c = ig + fc adds.

MLP for block b runs during block b+1's phase 1 (Pf tag is free there);
b2 is added on the host after gathering.
"""
import os
import numpy as np
import ml_dtypes

import concourse.bacc as bacc
import concourse.bass as bass
import concourse.mybir as mybir
import concourse.tile as tile
from concourse import bass_utils

F32 = mybir.dt.float32
BF16 = mybir.dt.bfloat16
AF = mybir.ActivationFunctionType

NCORES = 8
W = 512
ITERS = int(os.environ.get("K_ITERS", "64"))
BCORE = ITERS * 2 * W          # 65536 at full size
BATCH = BCORE * NCORES
BLK = int(os.environ.get("K_BLK", "8"))
NBLK = ITERS // BLK
BW = BLK * W                   # 4096: A(or B)-half width per block
XW2 = 2 * ITERS * W            # 65536: x columns per core

STATE_DIM, SEQ_LEN, HIDDEN, MLP_HIDDEN = 18, 3, 64, 32
KX = STATE_DIM + 2             # x rows + 2 ones rows (bias hi/lo)

# tanh deg-5 odd minimax on [-1.35,1.35] (for tanh(c)), maxerr 1.9e-3
T5H = (0.9892881449596069, -0.2766816650158005, 0.049318986652270407)
# tanh(x/2) deg-5 odd minimax on [-3.2,3.2] (for 2*sigmoid(f)), maxerr 4.2e-3
TF5 = (0.4896413693764065, -0.031307111156723334, 0.0011471116956348172)
# tanh deg-5 odd minimax on [-2.2,2.2] (for tanh(g_pre)), maxerr 1.5e-2
T25 = (0.9414658776605921, -0.1885272303414886, 0.017991445957406323)

KLAD = (0.25, 0.5, 1.0)        # running c-scale per step


def _hk(k):
    """tanh(c) evaluated on chat = k*c: chat*(c0/k + c1*chat^2/k^3 + ...)."""
    return (T5H[0] / k, T5H[1] / k ** 3, T5H[2] / k ** 5)


def _tk(k):
    """k * tanh5(g_pre) coefficients."""
    return (T25[0] * k, T25[1] * k, T25[2] * k)


_ops = None


def get_ops():
    """Register the two custom DVE ops (idempotent)."""
    global _ops
    if _ops is not None:
        return _ops
    import concourse.dve_ops as dve_ops
    from concourse.dve_spec import Spec, Src0, Src1, C0, C1, C2, One, sq, lower
    from concourse.dve_uop import DveOpSpec

    def make(name, body, ref):
        for op in dve_ops.OPS:
            if op.name == name:
                return op
        spec = Spec(body=body, reference=ref)
        if name not in dve_ops._SUB_OPCODE_FOR_NAME:
            dve_ops._SUB_OPCODE_FOR_NAME[name] = (
                max(dve_ops._SUB_OPCODE_FOR_NAME.values()) + 1)
        shas = {}
        for ver in ("v3", "v4"):
            try:
                s = DveOpSpec(name=name, opcode=dve_ops.get_dve_sub_opcode(name),
                              uops=lower(spec, ver=ver), rd1_en=True)
                shas[ver] = s.sha(ver)
            except Exception:
                pass
        op = dve_ops.DveOp(name, spec, subdim=False, uops_sha=shas)
        dve_ops.OPS.append(op)
        return op

    u = sq(Src0)
    poly = ((C2 * u + C1) * u + C0) * Src0

    def _ref_tm(in0, in1, s0, s1, imm2):
        x = in0.astype(np.float32)
        uu = x * x
        return ((imm2 * uu + s1) * uu + s0) * x * in1.astype(np.float32)

    def _ref_fs(in0, in1, s0, s1, imm2):
        x = in0.astype(np.float32)
        uu = x * x
        return (((imm2 * uu + s1) * uu + s0) * x + 1.0) * in1.astype(np.float32)

    tanhmul = make("TANHMUL5_ANT", poly * Src1, _ref_tm)
    sigscale = make("SIGSCALE5_ANT", (poly + One) * Src1, _ref_fs)
    _ops = (tanhmul, sigscale)
    return _ops


def build_bass():
    tanhmul, sigscale = get_ops()
    nc = bacc.Bacc("TRN2", target_bir_lowering=False, debug=False)

    W2_ = 2 * W

    xs_d = [nc.dram_tensor(f"xs{t}", [KX, XW2], BF16, kind="ExternalInput").ap()
            for t in range(SEQ_LEN)]
    wxq_d = nc.dram_tensor("wxq", [128, 64], BF16, kind="ExternalInput").ap()
    wh_d = {g: nc.dram_tensor(f"wh{g}", [128, 128], BF16,
                              kind="ExternalInput").ap() for g in "iogf"}
    w1_d = nc.dram_tensor("w1e", [128, 64], BF16, kind="ExternalInput").ap()
    w2_d = nc.dram_tensor("w2e", [64, 2], BF16, kind="ExternalInput").ap()
    b1_d = nc.dram_tensor("b1e", [64, 1], F32, kind="ExternalInput").ap()
    out_d = nc.dram_tensor("out", [2 * NBLK, BW], F32, kind="ExternalOutput").ap()

    # gate -> row-group of wxq / x quad-replication
    GR = {"i": 0, "o": 1, "g": 2, "f": 3}

    with tile.TileContext(nc) as tc:
        with tc.tile_pool(name="const", bufs=1) as cst, \
             tc.tile_pool(name="xt", bufs=2) as xtp, \
             tc.tile_pool(name="sg", bufs=4) as sgp, \
             tc.tile_pool(name="cw", bufs=6) as cwp, \
             tc.tile_pool(name="c3w", bufs=3) as c3p, \
             tc.tile_pool(name="hw", bufs=6) as hwp, \
             tc.tile_pool(name="h3", bufs=8) as h3p, \
             tc.tile_pool(name="igfc", bufs=3) as igp, \
             tc.tile_pool(name="zr", bufs=3) as zrp, \
             tc.tile_pool(name="vo", bufs=3) as vop, \
             tc.tile_pool(name="P3", bufs=2, space="PSUM") as p3p, \
             tc.tile_pool(name="Pf", bufs=2, space="PSUM") as pfp:

            wxq = cst.tile([128, 64], BF16)
            nc.sync.dma_start(wxq[:], wxq_d[:])
            wh = {}
            for g in "iogf":
                wh[g] = cst.tile([128, 128], BF16, name=f"wh{g}")
                nc.sync.dma_start(wh[g][:], wh_d[g][:])
            w1e = cst.tile([128, 64], BF16)
            nc.sync.dma_start(w1e[:], w1_d[:])
            w2e = cst.tile([64, 2], BF16)
            nc.sync.dma_start(w2e[:], w2_d[:])
            b1e = cst.tile([64, 1], F32)
            nc.sync.dma_start(b1e[:], b1_d[:])

            def x_mms(xt, j, gates, P3, Pf, stop):
                """x-projection for one j: concurrent subarray tiles."""
                cx = j * W
                dst = {"i": P3[:, 0:W], "o": P3[:, W:W2_],
                       "g": P3[:, W2_:3 * W], "f": Pf}
                for gate in gates:
                    r = GR[gate]
                    for c in (0, 1):
                        nc.tensor.matmul(
                            dst[gate][64 * c:64 * c + 64, :],
                            wxq[32 * r:32 * r + KX, 0:64],
                            xt[32 * r:32 * r + KX, c * BW + cx:c * BW + cx + W],
                            start=True, stop=stop,
                            tile_position=(32 * r, 64 * c),
                            skip_group_check=True)

            def h_mms(hpair, j, P3, Pf):
                """h-projection accumulate for one j (K=128 blockdiag)."""
                rhs = hpair[:, (j % 2) * W:(j % 2) * W + W]
                for gate, dst in (("i", P3[:, 0:W]), ("o", P3[:, W:W2_]),
                                  ("g", P3[:, W2_:3 * W]), ("f", Pf)):
                    nc.tensor.matmul(dst, wh[gate][:, :], rhs,
                                     start=False, stop=True,
                                     skip_group_check=True)

            def sig_act(Pio, sgt, j):
                """sigma over [i,o] -> sg pair tile strided halves."""
                p = j % 2
                src = Pio[:, 0:W2_].rearrange("p (g w) -> p g w", g=2, w=W)
                dst = sgt.rearrange("p (g j w) -> p g j w", g=2, j=2, w=W)[:, :, p, :]
                nc.scalar.activation(dst, src, AF.Sigmoid)

            def mlp(h3pair, blk, j):
                """MLP for (prev) block blk, iteration j; Pf tag is free in ph1."""
                Pz = pfp.tile([128, W], F32, tag="Pf", name="Pfm")
                nc.tensor.matmul(Pz[0:64, :], w1e[:, :],
                                 h3pair[:, (j % 2) * W:(j % 2) * W + W],
                                 start=True, stop=True, skip_group_check=True)
                zrt = zrp.tile([64, W], BF16, tag="zr")
                nc.scalar.activation(zrt[:], Pz[0:64, :], AF.Relu, bias=b1e[:])
                nc.tensor.matmul(Pz[96:98, :], w2e[:, :], zrt[:],
                                 start=True, stop=True, tile_position=(0, 96),
                                 skip_group_check=True)
                vot = vop.tile([2, W], F32, tag="vo")
                nc.scalar.copy(vot[:], Pz[96:98, :])
                nc.sync.dma_start(
                    out_d[2 * blk:2 * blk + 2, j * W:(j + 1) * W], vot[:])

            h3_prev = None
            for blk in range(NBLK):
                xt = []
                for t in range(SEQ_LEN):
                    xb = xtp.tile([128, 2 * BW], BF16, tag=f"x{t}")
                    for r in range(4):
                        nc.sync.dma_start(
                            xb[32 * r:32 * r + KX, :],
                            xs_d[t][:, blk * 2 * BW:(blk + 1) * 2 * BW])
                    xt.append(xb)

                # ---------- phase 1: step 1 (h0 = c0 = 0) + prev-block MLP ----
                c1s, h1s = [], []
                for j in range(BLK):
                    Pio = p3p.tile([128, 3 * W], F32, tag="P3", name="Pio1")
                    x_mms(xt[0], j, "iog", Pio, None, stop=True)
                    if j % 2 == 0:
                        sgt = sgp.tile([128, 4 * W], BF16, tag="sg", name="sg1")
                        c1t = cwp.tile([128, W2_], BF16, tag="c1", name="c1t")
                    sig_act(Pio, sgt, j)
                    if h3_prev is not None:
                        mlp(h3_prev[j // 2], blk - 1, j)
                    p = j % 2
                    nc.vector._custom_dve(
                        tanhmul, out=c1t[:, p * W:p * W + W],
                        in0=Pio[:, W2_:3 * W],
                        in1=sgt[:, p * W:p * W + W],
                        s0=_tk(KLAD[0])[0], s1=_tk(KLAD[0])[1],
                        imm2=_tk(KLAD[0])[2])
                    if j % 2 == 1:
                        h1t = hwp.tile([128, W2_], BF16, tag="h1", name="h1t")
                        hco = _hk(KLAD[0])
                        nc.vector._custom_dve(
                            tanhmul, out=h1t[:], in0=c1t[:],
                            in1=sgt[:, W2_:4 * W], s0=hco[0], s1=hco[1],
                            imm2=hco[2])
                        c1s.append(c1t)
                        h1s.append(h1t)

                # ---------- phase 2: step 2 ----------
                c2s, h2s = [], []
                for j in range(BLK):
                    Pio = p3p.tile([128, 3 * W], F32, tag="P3", name="Pio2")
                    Pf = pfp.tile([128, W], F32, tag="Pf", name="Pf2")
                    x_mms(xt[1], j, "iogf", Pio, Pf, stop=False)
                    h_mms(h1s[j // 2], j, Pio, Pf)
                    if j % 2 == 0:
                        sgt = sgp.tile([128, 4 * W], BF16, tag="sg", name="sg2")
                        c2t = cwp.tile([128, W2_], BF16, tag="c2", name="c2t")
                    p = j % 2
                    sig_act(Pio, sgt, j)
                    igt = igp.tile([128, W], BF16, tag="ig")
                    nc.vector._custom_dve(
                        tanhmul, out=igt[:], in0=Pio[:, W2_:3 * W],
                        in1=sgt[:, p * W:p * W + W],
                        s0=_tk(KLAD[1])[0], s1=_tk(KLAD[1])[1],
                        imm2=_tk(KLAD[1])[2])
                    fct = igp.tile([128, W], BF16, tag="fc")
                    nc.vector._custom_dve(
                        sigscale, out=fct[:], in0=Pf[:, :],
                        in1=c1s[j // 2][:, p * W:p * W + W],
                        s0=TF5[0], s1=TF5[1], imm2=TF5[2])
                    nc.gpsimd.tensor_add(c2t[:, p * W:p * W + W], igt[:], fct[:])
                    if j % 2 == 1:
                        h2t = hwp.tile([128, W2_], BF16, tag="h2", name="h2t")
                        hco = _hk(KLAD[1])
                        nc.vector._custom_dve(
                            tanhmul, out=h2t[:], in0=c2t[:],
                            in1=sgt[:, W2_:4 * W], s0=hco[0], s1=hco[1],
                            imm2=hco[2])
                        c2s.append(c2t)
                        h2s.append(h2t)

                # ---------- phase 3: step 3 ----------
                h3_cur = []
                for j in range(BLK):
                    Pio = p3p.tile([128, 3 * W], F32, tag="P3", name="Pio3")
                    Pf = pfp.tile([128, W], F32, tag="Pf", name="Pf3")
                    x_mms(xt[2], j, "iogf", Pio, Pf, stop=False)
                    h_mms(h2s[j // 2], j, Pio, Pf)
                    if j % 2 == 0:
                        sgt = sgp.tile([128, 4 * W], BF16, tag="sg", name="sg3")
                        c3t = c3p.tile([128, W2_], BF16, tag="c3", name="c3t")
                    p = j % 2
                    sig_act(Pio, sgt, j)
                    igt = igp.tile([128, W], BF16, tag="ig")
                    nc.vector._custom_dve(
                        tanhmul, out=igt[:], in0=Pio[:, W2_:3 * W],
                        in1=sgt[:, p * W:p * W + W],
                        s0=_tk(KLAD[2])[0], s1=_tk(KLAD[2])[1],
                        imm2=_tk(KLAD[2])[2])
                    fct = igp.tile([128, W], BF16, tag="fc")
                    nc.vector._custom_dve(
                        sigscale, out=fct[:], in0=Pf[:, :],
                        in1=c2s[j // 2][:, p * W:p * W + W],
                        s0=TF5[0], s1=TF5[1], imm2=TF5[2])
                    nc.gpsimd.tensor_add(c3t[:, p * W:p * W + W], igt[:], fct[:])
                    if j % 2 == 1:
                        h3t = h3p.tile([128, W2_], BF16, tag="h3", name="h3t")
                        hco = _hk(KLAD[2])
                        nc.vector._custom_dve(
                            tanhmul, out=h3t[:], in0=c3t[:],
                            in1=sgt[:, W2_:4 * W], s0=hco[0], s1=hco[1],
                            imm2=hco[2])
                        h3_cur.append(h3t)

                h3_prev = h3_cur

            # MLP for the last block
            for j in range(BLK):
                mlp(h3_prev[j // 2], NBLK - 1, j)

    nc.compile()
    return nc


def _host_prep(state_seq, W_ih, W_hh, b_ih, b_hh, W1, b1, W2, b2):
    """Build per-core input maps (host-side layout prep only)."""
    bf = ml_dtypes.bfloat16
    B = state_seq.shape[0]
    b = b_ih.astype(np.float64) + b_hh.astype(np.float64)  # [256]
    GROW = {"i": 0, "f": 64, "g": 128, "o": 192}           # torch gate order
    GR = {"i": 0, "o": 1, "g": 2, "f": 3}                  # row-group order

    wxq = np.zeros((128, 64), np.float64)
    for gate, r in GR.items():
        r0 = GROW[gate]
        Wx = W_ih[r0:r0 + 64, :].astype(np.float64)        # [64, 18]
        bg = b[r0:r0 + 64]
        b_hi = bg.astype(bf).astype(np.float64)
        b_lo = (bg - b_hi).astype(bf).astype(np.float64)
        wxq[32 * r:32 * r + STATE_DIM, :] = Wx.T
        wxq[32 * r + STATE_DIM, :] = b_hi
        wxq[32 * r + STATE_DIM + 1, :] = b_lo

    shared = {"wxq": wxq.astype(bf)}
    for gate in "iogf":
        r0 = GROW[gate]
        Wh = W_hh[r0:r0 + 64, :].astype(np.float64)        # [64, 64]
        whe = np.zeros((128, 128), np.float64)
        for half in range(2):
            o = half * 64
            whe[o:o + 64, o:o + 64] = Wh.T
        shared[f"wh{gate}"] = whe.astype(bf)

    w1e = np.zeros((128, 64), np.float64)
    w2e = np.zeros((64, 2), np.float64)
    b1e = np.zeros((64, 1), np.float32)
    for half in range(2):
        ro, co = half * 64, half * 32
        w1e[ro:ro + 64, co:co + 32] = W1.astype(np.float64).T
        w2e[co:co + 32, half] = W2[0].astype(np.float64)
        b1e[co:co + 32, 0] = b1
    shared["w1e"] = w1e.astype(bf)
    shared["w2e"] = w2e.astype(bf)
    shared["b1e"] = b1e

    # x: [KX, B_core] per step; ones rows 18,19; col = flat element index
    in_maps = []
    for cc in range(NCORES):
        lo = cc * BCORE
        m = dict(shared)
        for t in range(SEQ_LEN):
            a = np.ones((KX, BCORE), np.float32)
            a[0:STATE_DIM] = state_seq[lo:lo + BCORE, t, :].T
            m[f"xs{t}"] = a.astype(bf)
        in_maps.append(m)
    return in_maps


_cached = {}


def kernel(**inputs) -> np.ndarray:
    if "nc" not in _cached:
        _cached["nc"] = build_bass()
    nc = _cached["nc"]
    in_maps = _host_prep(**inputs)
    trace = bool(int(os.environ.get("K_TRACE", "0")))
    res = bass_utils.run_bass_kernel_spmd(nc, in_maps, core_ids=list(range(NCORES)),
                                          trace=trace)
    outs = [r["out"].reshape(-1) for r in res.results]
    _cached["last_results"] = res
    out = np.concatenate(outs).astype(np.float32)
    return out + np.float32(inputs["b2"][0])
